# revision 37
# baseline (speedup 1.0000x reference)
"""Trainium2 Bass kernel for nn_CustomLoss_45449343926664 (retrieval_knn).

loss = mse(mean(c1), mean(c2))
     + mean_i min_j ||c1_i - c2_j||^2
     + mean_k relu(0.1 - var(c1)_k)

v2 design (vs. the 76us v1):
  - Sharding: 4 i-blocks x 2 j-halves (each core: 2048 c1 rows x 4096 c2
    rows) so every PSUM drain instruction covers FD=2048 columns, halving
    the per-instruction overhead of the drain engines.
  - The [128 j, 2048 i] cross tiles (j on partitions, bias -sq2_j - C as a
    per-partition scalar) are drained by three engines in parallel:
      * DVE: fused scalar_tensor_tensor  z' = max(psum + bias, z)
      * ACT: activation(Identity, bias) -> bf16 z tiles; PAIRS of z tiles
        are max-folded into accumulators by DVE (bf16 2x tensor_tensor) or
        GPSIMD (int32-bitcast min: all z are < 0 after the C shift, so
        float max == int32 min on the raw bits).
  - mean/variance stats and |c1_i|^2 are computed on the host (tiny), so
    the device does nothing but the distance-max work.
  - Tail: combine accumulators, 16 PE transposes, one 3D reduce_max.

Host combine: gmax over the two j-halves, dist = mean(sq1 - gmax - C),
plus the host-computed mean/dispersion losses.
"""
import os
import sys

import numpy as np
import ml_dtypes

if os.path.isdir("/opt/trn_rl_repo") and "/opt/trn_rl_repo" not in sys.path:
    sys.path.insert(0, "/opt/trn_rl_repo")

from contextlib import ExitStack

import concourse.bass as bass
import concourse.tile as tile
from concourse import bacc, mybir
from concourse.bass_utils import run_bass_kernel_spmd
from concourse.masks import make_identity

F32 = mybir.dt.float32
BF16 = mybir.dt.bfloat16
I32 = mybir.dt.int32
U16 = mybir.dt.uint16
BF16_NP = ml_dtypes.bfloat16
NEG_BIG = -3.0e38

N_CORES = 8
N1 = 8192            # cluster1 rows (total)
N2 = 8192            # cluster2 rows
D = 128              # feature dim = partition count
P = 128
NIB = 4              # i-blocks
NJH = 2              # j-halves
IB = N1 // NIB       # 2048 c1 rows per core
JH = N2 // NJH       # 4096 c2 rows per core
NIT = IB // P        # 16 i-chunks of 128
NJT = JH // P        # 32 j-tiles of 128
NCHUNK = 4           # c2bT DMA chunks
JT_PER_CHUNK = NJT // NCHUNK   # 8

C_SHIFT = 64.0       # makes all z = 2<c1,c2> - |c2|^2 - C strictly negative

# j-tiles drained by the fused DVE scalar_tensor_tensor path; the rest go
# to ACT (activation w/ bias) + DVE bf16 pair-folds. (Pool/GPSIMD cannot
# run TensorTensor in this toolchain, so folds are DVE-only.)
# Work units are (j-tile, i-half) pairs of [128 j, 1024 i]; four of them fit
# PSUM simultaneously (4 x 2 banks), which keeps the fill/drain pipeline from
# binding. Drain routes per unit:
#   DVE:  fused scalar_tensor_tensor (bias+max) straight from PSUM (1x fp32)
#   ACT:  activation(Identity,+bias) -> bf16 z tiles; PAIRS fold into an
#         accumulator either on DVE (bf16 2x tensor_max) or via a
#         gpsimd-issued DMA with accum_op=max (SDMA CCE does the RMW,
#         costing idle DMA-rail bandwidth instead of DVE cycles).
DVE_T = frozenset({2, 4, 7, 10, 13, 15, 18, 21, 23, 26, 29, 31})  # 12/half
DMA_PAIRS = frozenset()            # SDMA CCE rejects accum_op=max
MM_N = 512           # max cols per matmul (fp32 PSUM out: one bank)
LDW_OPT = False      # walrus rejects ldw-opt with bass-lowered matmuls
HI = IB // 2         # 1024 i-cols per half

_cached = {}


def _patch_ldw_opt():
    """Flip walrus --enable-ldw-opt to true so back-to-back matmuls sharing
    one stationary operand don't reload the PE weight array each time (the
    reload serializes the pipeline: each MM runs at isolated (398+N)/2.4 ns
    instead of streaming at N/2.4)."""
    from concourse import bass_utils as BU

    if getattr(BU, "_ldw_opt_patched", False):
        return
    orig = BU.run_command

    def run_command_ldw(cmd, *a, **kw):
        if isinstance(cmd, list):
            cmd = [c.replace("--enable-ldw-opt=false", "--enable-ldw-opt=true")
                   if isinstance(c, str) else c for c in cmd]
        return orig(cmd, *a, **kw)

    BU.run_command = run_command_ldw
    BU._ldw_opt_patched = True


def _build_program():
    """Build + compile the single-core SPMD program (same for all cores)."""
    nc = bacc.Bacc(
        "TRN2",
        target_bir_lowering=False,
        debug=False,
        enable_asserts=False,
        num_devices=N_CORES,
    )

    d_c1bT = nc.dram_tensor("c1bT", [D, IB], BF16, kind="ExternalInput").ap()
    d_c2bT = nc.dram_tensor("c2bT", [D, JH], BF16, kind="ExternalInput").ap()
    d_sq2neg = nc.dram_tensor("sq2neg", [P, NJT], F32, kind="ExternalInput").ap()

    d_zfin = nc.dram_tensor("zfin", [P, IB], BF16, kind="ExternalOutput").ap()

    with tile.TileContext(nc) as tc, ExitStack() as ctx:
        const = ctx.enter_context(tc.tile_pool(name="const", bufs=1))
        c2pool = ctx.enter_context(tc.tile_pool(name="c2pool", bufs=NCHUNK))
        zring = ctx.enter_context(tc.tile_pool(name="zring", bufs=6))
        spool = ctx.enter_context(tc.tile_pool(name="spool", bufs=6))
        psum = ctx.enter_context(tc.tile_pool(name="psum", bufs=4, space="PSUM"))

        t_c1bT = const.tile([P, IB], BF16)
        t_sq2neg = const.tile([P, NJT], F32)
        t_zD = [const.tile([P, IB], BF16, name=f"zD{i}") for i in range(2)]
        t_zA = [const.tile([P, 2, IB], BF16, name=f"zA{i}") for i in range(2)]
        t_zM = const.tile([P, 2, IB], BF16, name="zM")   # DMA-fold accum
        t_zfin = const.tile([P, IB], BF16)
        t_ident = const.tile([P, P], BF16)
        t_ones = const.tile([P, 1], F32)
        t_dummy = const.tile([P, 1], F32)

        # ---- input DMAs first (chunk0 on gpsimd so it isn't queued behind
        # anything; first cross matmul needs chunk0 + c1bT) ----
        nc.sync.dma_start(t_c1bT[:], d_c1bT)
        t_c2bT = []
        dma_engs = [nc.gpsimd, nc.sync, nc.sync, nc.sync]
        for ci in range(NCHUNK):
            t = c2pool.tile([P, JT_PER_CHUNK, P], BF16, name=f"c2bT{ci}")
            dma_engs[ci].dma_start(
                t[:],
                d_c2bT[:, ci * JT_PER_CHUNK * P : (ci + 1) * JT_PER_CHUNK * P]
                .rearrange("k (t p) -> k t p", p=P),
            )
            t_c2bT.append(t)
        nc.gpsimd.dma_start(t_sq2neg[:], d_sq2neg)

        make_identity(nc, t_ident[:])
        nc.gpsimd.memset(t_zD[0][:], NEG_BIG)
        nc.gpsimd.memset(t_zA[0][:], NEG_BIG)
        if DMA_PAIRS:
            nc.gpsimd.memset(t_zM[:], NEG_BIG)
        nc.vector.memset(t_ones[:], 1.0)

        # warm the ACT table set before the drain path needs it
        nc.scalar.activation(t_dummy[:], t_ones[:],
                             mybir.ActivationFunctionType.Identity, bias=0.0)

        # PE warm-up: keep HAM busy while inputs stream in
        pwarm = psum.tile([P, P], F32, tag="pcross", name="pwarm")
        for w in range(12):
            nc.tensor.matmul(pwarm[:], t_ident[:], t_ident[:],
                             start=(w == 0), stop=(w == 11))

        # ---- cross matmuls (j on partitions) + DVE/ACT drain over
        # (j-tile, i-half) units of [128, 1024] ----
        nd = [0, 0]
        na = [0, 0]
        npair = [0, 0]
        zhalf = [0, 0]
        zt = [None, None]
        for t in range(NJT):
            lhsT = t_c2bT[t // JT_PER_CHUNK][:, t % JT_PER_CHUNK]
            bias = t_sq2neg[:, t : t + 1]
            for h in range(2):
                s = slice(h * HI, (h + 1) * HI)
                pt = psum.tile([P, HI], F32, tag="pcross", name="pcross")
                for c in range(HI // MM_N):
                    nc.tensor.matmul(
                        pt[:, c * MM_N : (c + 1) * MM_N],
                        lhsT,
                        t_c1bT[:, h * HI + c * MM_N : h * HI + (c + 1) * MM_N],
                        start=True, stop=True)
                if t in DVE_T:
                    nc.vector.scalar_tensor_tensor(
                        out=t_zD[(nd[h] + 1) % 2][:, s],
                        in0=pt[:],
                        scalar=bias,
                        in1=t_zD[nd[h] % 2][:, s],
                        op0=mybir.AluOpType.add,
                        op1=mybir.AluOpType.max,
                    )
                    nd[h] += 1
                else:
                    if zhalf[h] == 0:
                        zt[h] = zring.tile([P, 2, HI], BF16, name=f"zt{h}")
                    nc.scalar.activation(
                        zt[h][:, zhalf[h]], pt[:],
                        mybir.ActivationFunctionType.Identity,
                        bias=bias, scale=1.0,
                    )
                    if zhalf[h] == 1:
                        if npair[h] in DMA_PAIRS:
                            nc.gpsimd.dma_start(
                                t_zM[:, :, s], zt[h][:],
                                accum_op=mybir.AluOpType.max)
                        else:
                            nc.vector.tensor_max(
                                t_zA[(na[h] + 1) % 2][:, :, s],
                                t_zA[na[h] % 2][:, :, s], zt[h][:])
                            na[h] += 1
                        npair[h] += 1
                    zhalf[h] ^= 1

        # ---- tail: combine partial maxes per i-half, DMA out; the final
        # 128-lane (j) reduction happens on the host (2M bf16 total).
        t_zAh = const.tile([P, IB], BF16)
        for h in range(2):
            s = slice(h * HI, (h + 1) * HI)
            nc.vector.tensor_max(t_zAh[:, s], t_zA[na[h] % 2][:, 0, s],
                                 t_zA[na[h] % 2][:, 1, s])
            if DMA_PAIRS:
                nc.vector.tensor_max(t_zAh[:, s], t_zAh[:, s], t_zM[:, 0, s])
                nc.vector.tensor_max(t_zAh[:, s], t_zAh[:, s], t_zM[:, 1, s])
            nc.vector.tensor_max(t_zfin[:, s], t_zAh[:, s],
                                 t_zD[nd[h] % 2][:, s])
            nc.sync.dma_start(d_zfin[:, s], t_zfin[:, s])

    nc.compile()
    return nc


def _prep_inputs(cluster1: np.ndarray, cluster2: np.ndarray):
    """Host-side sharding + operand layout prep."""
    c2b = cluster2.astype(BF16_NP)
    c2bT = np.ascontiguousarray(c2b.T)                       # [128, 8192] bf16
    sq2 = (c2b.astype(np.float32) ** 2).sum(axis=1)          # [8192] fp32
    biasfull = (-(sq2 + C_SHIFT)).astype(np.float32)

    c1bTs = []
    for ib in range(NIB):
        blk = cluster1[ib * IB : (ib + 1) * IB]
        c1bTs.append(np.ascontiguousarray((2.0 * blk).astype(BF16_NP).T))

    in_maps = []
    for c in range(N_CORES):
        ib, jh = c // NJH, c % NJH
        sq2neg = np.ascontiguousarray(
            biasfull[jh * JH : (jh + 1) * JH].reshape(NJT, P).T)
        in_maps.append({
            "c1bT": c1bTs[ib],
            "c2bT": np.ascontiguousarray(c2bT[:, jh * JH : (jh + 1) * JH]),
            "sq2neg": sq2neg,
        })
    return in_maps


def _finish(cluster1, cluster2, results) -> np.float32:
    """Combine per-core gmax with host-side stats into the scalar loss."""
    c1 = np.asarray(cluster1, np.float64)
    c2 = np.asarray(cluster2, np.float64)

    dist_sum = 0.0
    for ib in range(NIB):
        g = None
        for jh in range(NJH):
            zf = np.asarray(results[ib * NJH + jh]["zfin"], np.float64)
            gm = zf.max(axis=0)                     # [2048] max over j-lanes
            g = gm if g is None else np.maximum(g, gm)
        maxz = g + C_SHIFT                          # [2048], indexed by i
        blk = c1[ib * IB : (ib + 1) * IB]
        sq1 = (blk * blk).sum(axis=1)
        dist_sum += (sq1 - maxz).sum()
    dist = dist_sum / N1

    m1 = c1.mean(axis=0)
    m2 = c2.mean(axis=0)
    mean_loss = ((m1 - m2) ** 2).mean()
    var = ((c1 - m1) ** 2).mean(axis=0)
    disp = np.maximum(0.1 - var, 0.0).mean()
    return np.float32(mean_loss + dist + disp)


def _run(inputs, trace=False, **kwargs):
    """Run on the 8 NeuronCores. Returns (loss_scalar, BassKernelResults)."""
    if LDW_OPT:
        _patch_ldw_opt()
    if "nc" not in _cached:
        _cached["nc"] = _build_program()
    nc = _cached["nc"]
    c1 = np.asarray(inputs["cluster1"], np.float32)
    c2 = np.asarray(inputs["cluster2"], np.float32)
    in_maps = _prep_inputs(c1, c2)
    res = run_bass_kernel_spmd(nc, in_maps, list(range(N_CORES)), trace=trace,
                               **kwargs)
    loss = _finish(c1, c2, res.results)
    return loss, res


def kernel(cluster1: np.ndarray, cluster2: np.ndarray) -> np.ndarray:
    loss, _ = _run({"cluster1": cluster1, "cluster2": cluster2})
    return np.asarray(loss, dtype=np.float32)
